# revision 1
# baseline (speedup 1.0000x reference)
"""Trainium2 Bass kernel for nn_MGCNLoss (segment_reduce).

Strategy (8 NeuronCores, SPMD):
  * Graph-sharded data parallelism: core c owns graphs [512c, 512(c+1)).
  * Host-side sharding step routes every node to its owning core and lays the
    core's nodes out as a fixed-stride padded matrix [512 graphs, PAD slots]
    (zero padding; PAD=2304 >= max nodes/graph). With that layout the on-device
    segment_sum is a dense per-partition row reduction (partition p of
    supertile s holds graph 512c+128s+p), the per-node normalization
    score/(sum[batch]+eps) is a per-partition broadcast, and the whole kernel
    is memory/DVE-bound as the problem's target_regime intends.
  * Device computes, per core: per-graph sums (segment_sum partials), their
    reciprocals, the per-node JS/KL terms (via ACT Ln + DVE fused
    multiply-accumulate), per-graph cross-entropy (max/exp/sum/log-softmax +
    one-hot target pick) and the correlation MSE, reduced to per-partition
    partials; partials are all-reduced across the 8 cores with a collective
    and every core computes the identical final (l_total, l_train, l_cor).

KL identity used (exactly the reference math, no approximation):
    sum_i [s_p*log((s_p+e)/(m+e)) + s_n*log((s_n+e)/(m+e))]
  = sum_i [s_p*Lp + s_n*Ln - (s_p+s_n)*Lm]
  with Lp=log(s_p+e), Ln=log(s_n+e), Lm=log(0.5*(s_p+s_n)+e)
  and sum_i s_p*Lp = r_p * sum_i x_i*Lp  (r_p is constant per graph/partition).
"""

import os

import numpy as np

import concourse.bass as bass
import concourse.bacc as bacc
import concourse.mybir as mybir
from concourse import tile
from concourse.bass_utils import run_bass_kernel_spmd

F32 = mybir.dt.float32
F16 = mybir.dt.float16
ALU = mybir.AluOpType
ACTF = mybir.ActivationFunctionType
AX = mybir.AxisListType

NUM_GRAPHS = 4096
NUM_NODES = 8_388_608
NUM_CLASSES = 10
NCORES = 8
GPC = NUM_GRAPHS // NCORES  # graphs per core = 512
ST = GPC // 128  # supertiles per core = 4
PAD = 2304  # padded slots per graph (actual max graph size is 2229)
NCH = 2  # chunks per supertile for pass 2
EPS = 1e-8
ALPHA = 1.0
BETA = 1.0
LAMBDA_COR = 0.1

LAST_RESULTS = None  # BassKernelResults of the most recent run (for test harness)


def _build_nc(pad: int, nch: int) -> bass.Bass:
    """Build the SPMD Bass program (identical on all 8 cores)."""
    del nch  # pass 2 runs full-width; kept in the signature as a cache key
    nc = bacc.Bacc(None, num_devices=NCORES)

    xp_d = nc.declare_dram_parameter("xp", [ST, 128, pad], F32, isOutput=False)
    xn_d = nc.declare_dram_parameter("xn", [ST, 128, pad], F32, isOutput=False)
    # meta: per graph row: [0:10]=logits, [10:20]=probs_pos, [20:30]=probs_neg,
    # [30]=target (as f32), [31]=zero pad
    mt_d = nc.declare_dram_parameter("mt", [ST, 128, 32], F32, isOutput=False)
    out_d = nc.declare_dram_parameter("out", [1, 3], F32, isOutput=True)

    iota_np = np.tile(np.arange(NUM_CLASSES, dtype=np.float32), (128, 1))
    iota_d = nc.inline_tensor(iota_np, name="iota10")

    with tile.TileContext(nc) as tc:
        with (
            tc.tile_pool(name="data", bufs=4) as dpool,
            tc.tile_pool(name="chunk", bufs=3) as cpool,
            tc.tile_pool(name="small", bufs=2) as spool,
            tc.tile_pool(name="persist", bufs=1) as ppool,
            tc.tile_pool(name="psum", bufs=1, space="PSUM") as pspool,
            tc.tile_pool(name="dram", bufs=1, space="DRAM") as drpool,
        ):
            iota_t = ppool.tile([128, NUM_CLASSES], F32)
            nc.sync.dma_start(iota_t[:], iota_d[:])
            # eps constant, produced on DVE so ACT ops reading it alongside
            # rp/rn (also DVE) need only one cross-engine wait
            eps_t = ppool.tile([128, 1], F32)
            nc.vector.tensor_scalar(
                eps_t[:], iota_t[:, 0:1], 0.0, EPS, op0=ALU.mult, op1=ALU.add
            )


            # per-supertile partial columns (persist across the loop)
            klc = ppool.tile([128, ST], F32)
            nzc = ppool.tile([128, ST], F32)
            cec = ppool.tile([128, ST], F32)
            msec = ppool.tile([128, ST], F32)

            for s in range(ST):
                # split each load in halves so pass-1 starts on the first half
                xp_t = dpool.tile([128, pad], F32, tag="xp")
                xn_t = dpool.tile([128, pad], F32, tag="xn")
                hf = pad // 2
                nc.sync.dma_start(xn_t[:, :hf], xn_d[s][:, :hf])
                nc.sync.dma_start(xp_t[:, :hf], xp_d[s][:, :hf])
                nc.sync.dma_start(xn_t[:, hf:], xn_d[s][:, hf:])
                nc.sync.dma_start(xp_t[:, hf:], xp_d[s][:, hf:])

                # ---- pass 1: per-graph sums (both on ACT copy-accum; the
                # fp16 copy outputs land in lp/ln and are overwritten by the
                # Ln activations below, same engine so just program order) ----
                lp_t = cpool.tile([128, pad], F16, tag="lp16")
                ln_t = cpool.tile([128, pad], F16, tag="ln16")
                spp = spool.tile([128, 2], F32, tag="spp")
                snp = spool.tile([128, 2], F32, tag="snp")
                for k in range(2):
                    sl = np.s_[:, k * hf : (k + 1) * hf]
                    nc.scalar.activation(
                        ln_t[sl], xn_t[sl], ACTF.Copy, accum_out=snp[:, k : k + 1]
                    )
                    nc.scalar.activation(
                        lp_t[sl], xp_t[sl], ACTF.Copy, accum_out=spp[:, k : k + 1]
                    )
                sp = spool.tile([128, 1], F32, tag="sp")
                nc.vector.tensor_tensor(sp[:], spp[:, 0:1], spp[:, 1:2], op=ALU.add)
                sn = spool.tile([128, 1], F32, tag="snn")
                nc.vector.tensor_tensor(sn[:], snp[:, 0:1], snp[:, 1:2], op=ALU.add)

                # non-empty graph indicator (counts>0 <=> sum of scores > 0)
                nc.vector.tensor_scalar(
                    nzc[:, s : s + 1], sp[:], 0.0, 0.0, op0=ALU.is_gt, op1=ALU.bypass
                )

                spe = spool.tile([128, 1], F32, tag="spe")
                nc.vector.tensor_scalar(
                    spe[:], sp[:], EPS, 0.0, op0=ALU.add, op1=ALU.bypass
                )
                rp = spool.tile([128, 1], F32, tag="rp")
                nc.vector.reciprocal(rp[:], spe[:])
                sne = spool.tile([128, 1], F32, tag="sne")
                nc.vector.tensor_scalar(
                    sne[:], sn[:], EPS, 0.0, op0=ALU.add, op1=ALU.bypass
                )
                rn = spool.tile([128, 1], F32, tag="rn")
                nc.vector.reciprocal(rn[:], sne[:])

                # ---- pass 2: KL terms ----
                # w via fused affine_then_add; the three product-sums via
                # fused affine_mul_reduce with fp32 accumulators (sp is never
                # materialised - its per-graph scale rides the fused op)
                aPs = spool.tile([128, 1], F32, tag="aPs")
                aNs = spool.tile([128, 1], F32, tag="aNs")
                aTs = spool.tile([128, 1], F32, tag="aTs")

                sn_t = cpool.tile([128, pad], F16, tag="sn16")
                nc.vector.tensor_scalar(
                    sn_t[:], xn_t[:], rn[:], 0.0, op0=ALU.mult, op1=ALU.bypass
                )
                w_t = cpool.tile([128, pad], F16, tag="w16")
                nc.vector.affine_then_add(
                    w_t[:], xp_t[:], sn_t[:], scale=rp[:], bias=0.0
                )
                nc.scalar.activation(
                    lp_t[:], xp_t[:], ACTF.Ln, bias=eps_t[:], scale=rp[:]
                )
                nc.scalar.activation(
                    ln_t[:], xn_t[:], ACTF.Ln, bias=eps_t[:], scale=rn[:]
                )
                lm_t = cpool.tile([128, pad], F16, tag="lm16")
                nc.scalar.activation(
                    lm_t[:], w_t[:], ACTF.Ln, bias=eps_t[:], scale=0.5
                )
                scr_t = cpool.tile([128, pad], F16, tag="scr16")
                nc.vector.affine_mul_reduce(
                    scr_t[:], aPs[:], xp_t[:], lp_t[:], scale=rp[:], bias=0.0
                )
                scr2_t = cpool.tile([128, pad], F16, tag="scr16")
                nc.vector.affine_mul_reduce(
                    scr2_t[:], aNs[:], sn_t[:], ln_t[:], scale=1.0, bias=0.0
                )
                scr3_t = cpool.tile([128, pad], F16, tag="scr16")
                nc.vector.affine_mul_reduce(
                    scr3_t[:], aTs[:], w_t[:], lm_t[:], scale=1.0, bias=0.0
                )

                # klc[:, s] = aPs + aNs - aTs
                t2 = spool.tile([128, 1], F32, tag="t2")
                nc.vector.tensor_tensor(t2[:], aPs[:], aNs[:], op=ALU.add)
                nc.vector.tensor_tensor(
                    klc[:, s : s + 1], t2[:], aTs[:], op=ALU.subtract
                )

                # ---- CE + MSE for this supertile's 128 graphs ----
                mt_t = spool.tile([128, 32], F32, tag="mt")
                nc.sync.dma_start(mt_t[:], mt_d[s])
                lg = mt_t[:, 0:NUM_CLASSES]
                pp = mt_t[:, NUM_CLASSES : 2 * NUM_CLASSES]
                pn = mt_t[:, 2 * NUM_CLASSES : 3 * NUM_CLASSES]
                tgf = mt_t[:, 30:31]

                mx = spool.tile([128, 1], F32, tag="mx")
                nc.vector.reduce_max(mx[:], lg, axis=AX.X)
                negm = spool.tile([128, 1], F32, tag="negm")
                nc.vector.tensor_scalar(
                    negm[:], mx[:], -1.0, 0.0, op0=ALU.mult, op1=ALU.bypass
                )
                e_t = spool.tile([128, NUM_CLASSES], F32, tag="e")
                nc.scalar.activation(e_t[:], lg, ACTF.Exp, bias=negm[:])
                s1 = spool.tile([128, 1], F32, tag="s1")
                nc.vector.reduce_sum(s1[:], e_t[:], axis=AX.X)
                ls = spool.tile([128, 1], F32, tag="ls")
                nc.scalar.activation(ls[:], s1[:], ACTF.Ln)
                lse = spool.tile([128, 1], F32, tag="lse")
                nc.vector.tensor_tensor(lse[:], ls[:], mx[:], op=ALU.add)
                oh = spool.tile([128, NUM_CLASSES], F32, tag="oh")
                nc.vector.tensor_tensor(
                    oh[:], iota_t[:], tgf.to_broadcast([128, NUM_CLASSES]),
                    op=ALU.is_equal,
                )
                ohs = spool.tile([128, NUM_CLASSES], F32, tag="ohs")
                pick = spool.tile([128, 1], F32, tag="pick")
                nc.vector.scalar_tensor_tensor(
                    ohs[:], oh[:], 1.0, lg, op0=ALU.bypass, op1=ALU.mult,
                    accum_out=pick[:],
                )
                nc.vector.tensor_tensor(
                    cec[:, s : s + 1], lse[:], pick[:], op=ALU.subtract
                )

                d_t = spool.tile([128, NUM_CLASSES], F32, tag="d")
                nc.vector.scalar_tensor_tensor(
                    d_t[:], pp, 1.0, pn, op0=ALU.subtract, op1=ALU.add
                )
                d2_t = spool.tile([128, NUM_CLASSES], F32, tag="d2")
                nc.vector.scalar_tensor_tensor(
                    d2_t[:], d_t[:], 1.0, d_t[:], op0=ALU.bypass, op1=ALU.mult,
                    accum_out=msec[:, s : s + 1],
                )

            # ---- fold the 4 supertile columns, stack into [128, 4] partials ----
            par = ppool.tile([128, 4], F32)
            nc.vector.reduce_sum(par[:, 0:1], klc[:], axis=AX.X)
            nc.vector.reduce_sum(par[:, 1:2], nzc[:], axis=AX.X)
            nc.vector.reduce_sum(par[:, 2:3], cec[:], axis=AX.X)
            nc.vector.reduce_sum(par[:, 3:4], msec[:], axis=AX.X)

            # ---- partition-reduce partials on PE, then a [1,4] AllReduce ----
            ones_t = ppool.tile([128, 1], F32)
            nc.vector.tensor_scalar(
                ones_t[:], iota_t[:, 0:1], 0.0, 1.0, op0=ALU.mult, op1=ALU.add
            )
            par_ps = pspool.tile([1, 4], F32)
            nc.tensor.matmul(
                par_ps[:], lhsT=ones_t[:], rhs=par[:], start=True, stop=True
            )
            par1 = ppool.tile([1, 4], F32)
            nc.vector.tensor_copy(par1[:], par_ps[:])
            cc_in = drpool.tile([1, 4], F32)
            nc.sync.dma_start(cc_in[:], par1[:])
            cc_out = drpool.tile([1, 4], F32)
            nc.gpsimd.collective_compute(
                "AllReduce",
                ALU.add,
                replica_groups=[list(range(NCORES))],
                ins=[cc_in.opt()],
                outs=[cc_out.opt()],
            )
            allp4 = ppool.tile([1, 4], F32)
            nc.sync.dma_start(allp4[:], cc_out[:])

            # ---- final scalar math (identical on every core) ----
            kl_s = allp4[:, 0:1]
            ng_s = allp4[:, 1:2]
            ce_s = allp4[:, 2:3]
            ms_s = allp4[:, 3:4]

            rng = ppool.tile([1, 1], F32)
            nc.vector.reciprocal(rng[:], ng_s)
            tj = ppool.tile([1, 1], F32)
            nc.vector.tensor_tensor(tj[:], kl_s, rng[:], op=ALU.mult)
            js = ppool.tile([1, 1], F32)
            nc.vector.tensor_scalar(
                js[:], tj[:], 0.5 * ALPHA, 0.0, op0=ALU.mult, op1=ALU.bypass
            )
            lcor = ppool.tile([1, 1], F32)
            nc.vector.scalar_tensor_tensor(
                lcor[:], ms_s, BETA / (NUM_GRAPHS * NUM_CLASSES), js[:],
                op0=ALU.mult, op1=ALU.add,
            )
            ltr = ppool.tile([1, 1], F32)
            nc.vector.tensor_scalar(
                ltr[:], ce_s, 1.0 / NUM_GRAPHS, 0.0, op0=ALU.mult, op1=ALU.bypass
            )
            ltot = ppool.tile([1, 1], F32)
            nc.vector.scalar_tensor_tensor(
                ltot[:], lcor[:], LAMBDA_COR, ltr[:], op0=ALU.mult, op1=ALU.add
            )

            outv = ppool.tile([1, 3], F32)
            nc.vector.tensor_copy(outv[:, 0:1], ltot[:])
            nc.vector.tensor_copy(outv[:, 1:2], ltr[:])
            nc.vector.tensor_copy(outv[:, 2:3], lcor[:])
            nc.sync.dma_start(out_d[:], outv[:])

    nc.finalize()
    return nc


def _pack_host(score_pos, score_neg, batch, pad):
    """Group nodes by graph into a zero-padded [NUM_GRAPHS, pad] layout."""
    n = batch.shape[0]
    counts = np.bincount(batch, minlength=NUM_GRAPHS)
    assert counts.max() <= pad, f"graph size {counts.max()} exceeds pad {pad}"
    order = np.argsort(batch, kind="stable")
    bs = batch[order]
    starts = np.zeros(NUM_GRAPHS, np.int64)
    starts[1:] = np.cumsum(counts)[:-1]
    pos = np.arange(n, dtype=np.int64) - starts[bs]
    xp = np.zeros((NUM_GRAPHS, pad), np.float32)
    xn = np.zeros((NUM_GRAPHS, pad), np.float32)
    xp[bs, pos] = np.asarray(score_pos, np.float32)[order]
    xn[bs, pos] = np.asarray(score_neg, np.float32)[order]
    return xp, xn


_NC_CACHE: dict = {}


def kernel(logits_pos, probs_pos, probs_neg, score_pos, score_neg, targets, batch):
    global LAST_RESULTS
    logits_pos = np.asarray(logits_pos, np.float32)
    probs_pos = np.asarray(probs_pos, np.float32)
    probs_neg = np.asarray(probs_neg, np.float32)
    score_pos = np.asarray(score_pos, np.float32)
    score_neg = np.asarray(score_neg, np.float32)
    targets = np.asarray(targets)
    batch = np.asarray(batch)

    # --- host-side sharding: route nodes to the core owning their graph,
    # grouped by graph with zero padding to a fixed stride ---
    xp, xn = _pack_host(score_pos, score_neg, batch, PAD)
    xp_c = xp.reshape(NCORES, ST, 128, PAD)
    xn_c = xn.reshape(NCORES, ST, 128, PAD)
    mt = np.concatenate(
        [
            logits_pos.reshape(NCORES, ST, 128, NUM_CLASSES),
            probs_pos.reshape(NCORES, ST, 128, NUM_CLASSES),
            probs_neg.reshape(NCORES, ST, 128, NUM_CLASSES),
            targets.astype(np.float32).reshape(NCORES, ST, 128, 1),
            np.zeros((NCORES, ST, 128, 1), np.float32),
        ],
        axis=-1,
    )

    key = (PAD, NCH)
    if key not in _NC_CACHE:
        _NC_CACHE[key] = _build_nc(PAD, NCH)
    nc = _NC_CACHE[key]

    in_maps = [
        {"xp": xp_c[c], "xn": xn_c[c], "mt": mt[c]} for c in range(NCORES)
    ]
    trace = bool(int(os.environ.get("KERNEL_TRACE", "0")))
    res = run_bass_kernel_spmd(nc, in_maps, list(range(NCORES)), trace=trace)
    LAST_RESULTS = res
    out = np.asarray(res.results[0]["out"], np.float32).reshape(3)
    return (np.float32(out[0]), np.float32(out[1]), np.float32(out[2]))



# revision 2
# speedup vs baseline: 1.4821x; 1.4821x over previous
"""Trainium2 Bass kernel for nn_MGCNLoss (segment_reduce), v2.

Strategy (8 NeuronCores, SPMD, no collective):
  * Graph-sharded data parallelism: core c owns graphs [512c, 512(c+1)),
    supertile s of core c holds graphs 512c+128s+p on partition p.
  * Host packs each graph's nodes into a fixed-stride fp16 row (zero padded to
    PAD slots); xp and xn ride one joint [128, 2*PAD] tile per supertile so the
    big Ln activation runs once over both.
  * Raw-value log identity: sum_i s*log(s+e) = r*sum_i x*log(x+e') +
    (r*S)*log(r) with s = r*x, r = 1/(S+e). The device therefore computes only
    raw-value quantities per graph -- S_p, S_n, A_p = sum x_p*ln(x_p+e),
    A_n = sum x_n*ln(x_n+e), A_m = sum y*ln(y+e) with y = x_p + rho*x_n,
    rho = (S_p+e)/(S_n+e) -- plus CE/MSE partials. The tiny per-graph
    fixups and the 8-way reduction happen on the host in float64 as part of
    the unshard step (replaces the trailing AllReduce).
  * Engine budget per supertile: ACT = one Ln over [128,2*PAD] + one Ln over
    [128,PAD] (only Ln tables -> no table thrash; CE's Exp runs first).
    DVE = two tensor_scalar+accum sums (4x fp16 mode), one
    scalar_tensor_tensor for y, three scalar_tensor_tensor product+accums.
"""

import os

import numpy as np

import concourse.bass as bass
import concourse.bacc as bacc
import concourse.mybir as mybir
from concourse import tile
from concourse.bass_utils import run_bass_kernel_spmd

F32 = mybir.dt.float32
F16 = mybir.dt.float16
ALU = mybir.AluOpType
ACTF = mybir.ActivationFunctionType
AX = mybir.AxisListType

NUM_GRAPHS = 4096
NUM_NODES = 8_388_608
NUM_CLASSES = 10
NCORES = 8
GPC = NUM_GRAPHS // NCORES  # graphs per core = 512
ST = GPC // 128  # supertiles per core = 4
PAD = 2232  # padded slots per graph (actual max graph size is 2229)
EPS = 1e-8
ALPHA = 1.0
BETA = 1.0
LAMBDA_COR = 0.1

# output column layout: [0:4]=A_p, [4:8]=A_n, [8:12]=A_m, [12:16]=S_p,
# [16:20]=S_n, [20]=ce partial, [21]=mse partial
OCOLS = 22

LAST_RESULTS = None  # BassKernelResults of the most recent run (for harness)


def _build_nc(pad: int) -> bass.Bass:
    """Build the SPMD Bass program (identical on all 8 cores)."""
    nc = bacc.Bacc(None, num_devices=NCORES)

    xpn_d = nc.declare_dram_parameter("xpn", [ST, 128, 2 * pad], F16, isOutput=False)
    # meta per core: [0:40]=logits (st-major), [40:80]=probs_pos,
    # [80:120]=probs_neg, [120:160]=target broadcast over 10 cols
    mt_d = nc.declare_dram_parameter("mt", [128, 160], F32, isOutput=False)
    out_d = nc.declare_dram_parameter("out", [128, OCOLS], F32, isOutput=True)

    # consts: cols 0:40 = iota10 tiled 4x, col 40 = EPS
    cst_np = np.concatenate(
        [
            np.tile(np.arange(NUM_CLASSES, dtype=np.float32), (128, ST)),
            np.full((128, 1), EPS, np.float32),
        ],
        axis=1,
    )
    cst_d = nc.inline_tensor(cst_np, name="cst41")

    with tile.TileContext(nc) as tc:
        with (
            tc.tile_pool(name="data", bufs=4) as dpool,
            tc.tile_pool(name="logs", bufs=2) as lpool,
            tc.tile_pool(name="mid", bufs=2) as ypool,
            tc.tile_pool(name="scr", bufs=3) as cpool,
            tc.tile_pool(name="small", bufs=2) as spool,
            tc.tile_pool(name="persist", bufs=1) as ppool,
        ):
            cst_t = ppool.tile([128, 41], F32)
            nc.sync.dma_start(cst_t[:], cst_d[:])
            mt_t = ppool.tile([128, 160], F32)
            nc.sync.dma_start(mt_t[:], mt_d[:])
            iota_t = cst_t[:, 0:40]
            eps_t = cst_t[:, 40:41]

            outv = ppool.tile([128, OCOLS], F32)

            # start the supertile loads right away (round-robins DMA queues)
            xpn_ts = []
            for s in range(ST):
                xpn_t = dpool.tile([128, 2 * pad], F16, tag="xpn")
                nc.sync.dma_start(xpn_t[:], xpn_d[s])
                xpn_ts.append(xpn_t)

            # ---- CE + MSE for this core's 512 graphs (batched, runs during
            # the first data load; Exp first so the Ln table loads once) ----
            lg = mt_t[:, 0:40]
            pp = mt_t[:, 40:80]
            pn = mt_t[:, 80:120]
            tgb = mt_t[:, 120:160]

            e_t = spool.tile([128, 40], F32, tag="e")
            nc.scalar.activation(e_t[:], lg, ACTF.Exp)
            s1 = spool.tile([128, ST], F32, tag="s1")
            for k in range(ST):
                nc.vector.reduce_sum(
                    s1[:, k : k + 1], e_t[:, 10 * k : 10 * k + 10], axis=AX.X
                )
            ls4 = spool.tile([128, ST], F32, tag="ls4")
            lse_p = spool.tile([128, 1], F32, tag="lse")
            nc.scalar.activation(ls4[:], s1[:], ACTF.Ln, accum_out=lse_p[:])
            oh = spool.tile([128, 40], F32, tag="oh")
            nc.vector.tensor_tensor(oh[:], iota_t, tgb, op=ALU.is_equal)
            ohs = spool.tile([128, 40], F32, tag="ohs")
            pick_p = spool.tile([128, 1], F32, tag="pick")
            nc.vector.scalar_tensor_tensor(
                ohs[:], oh[:], 1.0, lg, op0=ALU.mult, op1=ALU.mult,
                accum_out=pick_p[:],
            )
            nc.vector.tensor_tensor(
                outv[:, 20:21], lse_p[:], pick_p[:], op=ALU.subtract
            )
            d_t = spool.tile([128, 40], F32, tag="d")
            nc.vector.scalar_tensor_tensor(
                d_t[:], pp, -1.0, pn, op0=ALU.add, op1=ALU.add
            )
            d2_t = spool.tile([128, 40], F32, tag="d2")
            nc.vector.scalar_tensor_tensor(
                d2_t[:], d_t[:], 1.0, d_t[:], op0=ALU.mult, op1=ALU.mult,
                accum_out=outv[:, 21:22],
            )

            # ---- main loop over supertiles ----
            for s in range(ST):
                xpn_t = xpn_ts[s]
                xp = xpn_t[:, 0:pad]
                xn = xpn_t[:, pad : 2 * pad]

                # raw per-graph sums S_p, S_n (fp16 4x tensor_scalar + accum)
                scr_a = cpool.tile([128, pad], F16, tag="scr")
                nc.vector.tensor_scalar(
                    scr_a[:], xp, 1.0, 0.0, op0=ALU.mult, op1=ALU.add,
                    accum_out=outv[:, 12 + s : 13 + s],
                )
                scr_b = cpool.tile([128, pad], F16, tag="scr")
                nc.vector.tensor_scalar(
                    scr_b[:], xn, 1.0, 0.0, op0=ALU.mult, op1=ALU.add,
                    accum_out=outv[:, 16 + s : 17 + s],
                )

                # rho = (S_p+e)/(S_n+e)
                t1 = spool.tile([128, 1], F32, tag="t1")
                nc.vector.tensor_scalar(
                    t1[:], outv[:, 12 + s : 13 + s], EPS, 0.0,
                    op0=ALU.add, op1=ALU.add,
                )
                t2 = spool.tile([128, 1], F32, tag="t2")
                nc.vector.tensor_scalar(
                    t2[:], outv[:, 16 + s : 17 + s], EPS, 0.0,
                    op0=ALU.add, op1=ALU.add,
                )
                t3 = spool.tile([128, 1], F32, tag="t3")
                nc.vector.reciprocal(t3[:], t2[:])
                rho = spool.tile([128, 1], F32, tag="rho")
                nc.vector.tensor_tensor(rho[:], t1[:], t3[:], op=ALU.mult)

                # joint log pass over (xp | xn)
                lpn_t = lpool.tile([128, 2 * pad], F16, tag="lpn")
                nc.scalar.activation(
                    lpn_t[:], xpn_t[:], ACTF.Ln, bias=eps_t, scale=1.0
                )

                # y = xp + rho*xn, then its log
                y_t = ypool.tile([128, pad], F16, tag="y")
                nc.vector.scalar_tensor_tensor(
                    y_t[:], xn, rho[:], xp, op0=ALU.mult, op1=ALU.add
                )
                lm_t = ypool.tile([128, pad], F16, tag="lm")
                nc.scalar.activation(
                    lm_t[:], y_t[:], ACTF.Ln, bias=eps_t, scale=1.0
                )

                # product accumulations A_p, A_n, A_m
                scr_c = cpool.tile([128, pad], F16, tag="scr")
                nc.vector.scalar_tensor_tensor(
                    scr_c[:], xp, 1.0, lpn_t[:, 0:pad], op0=ALU.mult,
                    op1=ALU.mult, accum_out=outv[:, 0 + s : 1 + s],
                )
                scr_d = cpool.tile([128, pad], F16, tag="scr")
                nc.vector.scalar_tensor_tensor(
                    scr_d[:], xn, 1.0, lpn_t[:, pad : 2 * pad], op0=ALU.mult,
                    op1=ALU.mult, accum_out=outv[:, 4 + s : 5 + s],
                )
                scr_e = cpool.tile([128, pad], F16, tag="scr")
                nc.vector.scalar_tensor_tensor(
                    scr_e[:], y_t[:], 1.0, lm_t[:], op0=ALU.mult,
                    op1=ALU.mult, accum_out=outv[:, 8 + s : 9 + s],
                )

            nc.sync.dma_start(out_d[:], outv[:])

    nc.finalize()
    return nc


def _pack_host(score_pos, score_neg, batch, pad):
    """Group nodes by graph into a zero-padded [NUM_GRAPHS, pad] fp16 layout."""
    n = batch.shape[0]
    counts = np.bincount(batch, minlength=NUM_GRAPHS)
    assert counts.max() <= pad, f"graph size {counts.max()} exceeds pad {pad}"
    order = np.argsort(batch, kind="stable")
    bs = batch[order]
    starts = np.zeros(NUM_GRAPHS, np.int64)
    starts[1:] = np.cumsum(counts)[:-1]
    pos = np.arange(n, dtype=np.int64) - starts[bs]
    xp = np.zeros((NUM_GRAPHS, pad), np.float16)
    xn = np.zeros((NUM_GRAPHS, pad), np.float16)
    xp[bs, pos] = np.asarray(score_pos, np.float16)[order]
    xn[bs, pos] = np.asarray(score_neg, np.float16)[order]
    return xp, xn, counts


_NC_CACHE: dict = {}


def kernel(logits_pos, probs_pos, probs_neg, score_pos, score_neg, targets, batch):
    global LAST_RESULTS
    logits_pos = np.asarray(logits_pos, np.float32)
    probs_pos = np.asarray(probs_pos, np.float32)
    probs_neg = np.asarray(probs_neg, np.float32)
    score_pos = np.asarray(score_pos, np.float32)
    score_neg = np.asarray(score_neg, np.float32)
    targets = np.asarray(targets)
    batch = np.asarray(batch)

    # --- host-side sharding: route nodes to the core owning their graph ---
    counts = np.bincount(batch, minlength=NUM_GRAPHS)
    pad = PAD if counts.max() <= PAD else int(np.ceil(counts.max() / 8) * 8)
    xp, xn, counts = _pack_host(score_pos, score_neg, batch, pad)
    # joint per-supertile tile: [core, st, 128, 2*pad]
    xpn = np.concatenate(
        [
            xp.reshape(NCORES, ST, 128, pad),
            xn.reshape(NCORES, ST, 128, pad),
        ],
        axis=-1,
    )

    # meta, field-grouped st-major: [core, 128, 160]
    def fold(a):  # [4096, 10] -> [core, 128, 40]
        return (
            a.reshape(NCORES, ST, 128, NUM_CLASSES)
            .transpose(0, 2, 1, 3)
            .reshape(NCORES, 128, ST * NUM_CLASSES)
        )

    tgb = np.repeat(
        targets.astype(np.float32).reshape(NCORES, ST, 128, 1), NUM_CLASSES, axis=3
    )
    mt = np.concatenate(
        [fold(logits_pos), fold(probs_pos), fold(probs_neg), fold(tgb.reshape(NCORES, ST, 128, NUM_CLASSES))],
        axis=2,
    ).astype(np.float32)

    if pad not in _NC_CACHE:
        _NC_CACHE[pad] = _build_nc(pad)
    nc = _NC_CACHE[pad]

    in_maps = [{"xpn": xpn[c], "mt": mt[c]} for c in range(NCORES)]
    trace = bool(int(os.environ.get("KERNEL_TRACE", "0")))
    res = run_bass_kernel_spmd(nc, in_maps, list(range(NCORES)), trace=trace)
    LAST_RESULTS = res

    # --- host unshard: combine per-core per-graph partials in float64 ---
    out = np.stack(
        [np.asarray(res.results[c]["out"], np.float64) for c in range(NCORES)]
    )  # [core, 128, OCOLS]

    def graphs(colbase):  # [core, 128, 4] -> [4096] in graph order
        return out[:, :, colbase : colbase + ST].transpose(0, 2, 1).reshape(-1)

    A_p, A_n, A_m = graphs(0), graphs(4), graphs(8)
    S_p, S_n = graphs(12), graphs(16)

    rp = 1.0 / (S_p + EPS)
    rn = 1.0 / (S_n + EPS)
    rho = (S_p + EPS) / (S_n + EPS)
    P = rp * A_p + (rp * S_p) * np.log(rp)
    N = rn * A_n + (rn * S_n) * np.log(rn)
    Sy = S_p + rho * S_n
    M = rp * A_m + (rp * Sy) * np.log(0.5 * rp)
    kl = P + N - M

    num_graphs = float((counts > 0).sum())
    js = 0.5 * kl.sum() / num_graphs
    ce_total = out[:, :, 20].sum()
    mse_total = out[:, :, 21].sum()
    l_train = ce_total / NUM_GRAPHS
    mse = mse_total / (NUM_GRAPHS * NUM_CLASSES)
    l_cor = ALPHA * js + BETA * mse
    l_total = l_train + LAMBDA_COR * l_cor
    return (np.float32(l_total), np.float32(l_train), np.float32(l_cor))


# revision 9
# speedup vs baseline: 2.1327x; 1.4389x over previous
"""Trainium2 Bass kernel for nn_MGCNLoss (segment_reduce), v3.

Strategy (8 NeuronCores, SPMD, no collective):
  * Graph-sharded data parallelism: core c owns graphs [512c, 512(c+1)) in 4
    blocks of 128 graphs.
  * TRANSPOSED layout: node slot j = 128k + p of graph g lands on partition p,
    column 128k + g of a [128, 2304] fp16 tile (18 chunks of 128 columns).
    With slots on partitions, per-graph reductions become partition-axis
    contractions on the otherwise-idle TENSOR engine:
      - sums S = ones^T @ x, PSUM-accumulated over the 18 chunks
      - product-sums sum_j x*L = diag(x^T L), chunk-accumulated in PSUM and
        extracted with one scalar_tensor_tensor (identity mask + accum).
    This moves all O(N) reduce work off the DVE, whose accumulate-op variants
    only run at 1x rate (the v2 bottleneck: 84% DVE busy).
  * Raw-value log identity: sum_i s*log(s+e) = r*sum_i x*log(x+e') +
    (r*S)*log(r) with s = r*x, r = 1/(S+e). ACT does one joint Ln over
    (xp|xn) and one Ln over the mixture y = xp + rho*xn, where
    rho = (Sp+e)/(Sn+e) is broadcast to columns via K=1 matmuls into PSUM.
    The per-graph fixups and 8-way reduction happen on the host in float64 as
    part of the unshard step; the device ships its fp16 rho so host math uses
    exactly the mixture weights the device applied.
"""

import os

import numpy as np

import concourse.bass as bass
import concourse.bacc as bacc
import concourse.mybir as mybir
from concourse import tile
from concourse.bass_utils import run_bass_kernel_spmd

F32 = mybir.dt.float32
F16 = mybir.dt.float16
ALU = mybir.AluOpType
ACTF = mybir.ActivationFunctionType
AX = mybir.AxisListType

NUM_GRAPHS = 4096
NUM_CLASSES = 10
NCORES = 8
ST = 4  # graph-blocks per core (128 graphs each)
PAD = 2304  # padded slots per graph = 18 chunks of 128
EPS = 1e-8
ALPHA = 1.0
BETA = 1.0
LAMBDA_COR = 0.1

LAST_RESULTS = None  # BassKernelResults of the most recent run (for harness)


def _build_nc(pad: int) -> bass.Bass:
    nc = bacc.Bacc(None, num_devices=NCORES)
    nch = pad // 128

    xpn_d = nc.declare_dram_parameter("xpn", [ST, 128, 2 * pad], F16, isOutput=False)
    mt_d = nc.declare_dram_parameter("mt", [128, 160], F32, isOutput=False)
    # outA: [0:4]=A_p, [4:8]=A_n, [8:12]=A_m per block, [12]=ce, [13]=mse
    outa_d = nc.declare_dram_parameter("outA", [128, 14], F32, isOutput=True)
    # outS block b at [384b:384b+384]: [0:128]=S_p, [128:256]=S_n, [256:384]=rho
    outs_d = nc.declare_dram_parameter("outS", [1, ST * 384], F32, isOutput=True)

    cst_np = np.concatenate(
        [
            np.tile(np.arange(NUM_CLASSES, dtype=np.float32), (128, ST)),
            np.full((128, 1), EPS, np.float32),
        ],
        axis=1,
    )
    cst_d = nc.inline_tensor(cst_np, name="cst41")
    # fp16 consts: identity (diag extract), ones col (sum matmuls)
    iden_np = np.concatenate(
        [np.eye(128, dtype=np.float16), np.ones((128, 1), np.float16)], axis=1
    )
    iden_d = nc.inline_tensor(iden_np, name="iden129")
    onesr_d = nc.inline_tensor(np.ones((1, 128), np.float16), name="onesr")

    with tile.TileContext(nc) as tc:
        with (
            tc.tile_pool(name="data", bufs=4) as dpool,
            tc.tile_pool(name="logs", bufs=4) as lpool,
            tc.tile_pool(name="mid", bufs=4) as ypool,
            tc.tile_pool(name="scr", bufs=3) as cpool,
            tc.tile_pool(name="small", bufs=4) as spool,
            tc.tile_pool(name="persist", bufs=1) as ppool,
            tc.tile_pool(name="psS", bufs=1, space="PSUM") as pss,
            tc.tile_pool(name="psP", bufs=2, space="PSUM") as psp,
            tc.tile_pool(name="psR", bufs=1, space="PSUM") as psr,
        ):
            cst_t = ppool.tile([128, 41], F32)
            nc.sync.dma_start(cst_t[:], cst_d[:])
            mt_t = ppool.tile([128, 160], F32)
            nc.sync.dma_start(mt_t[:], mt_d[:])
            iden_t = ppool.tile([128, 129], F16)
            nc.sync.dma_start(iden_t[:], iden_d[:])
            onesr_t = ppool.tile([1, 128], F16)
            nc.sync.dma_start(onesr_t[:], onesr_d[:])
            iota_t = cst_t[:, 0:40]
            eps_t = cst_t[:, 40:41]
            onesc_t = iden_t[:, 128:129]

            outa_t = ppool.tile([128, 14], F32)
            outs_t = ppool.tile([1, ST * 384], F32)

            xpn_ts = []
            for b in range(ST):
                xpn_t = dpool.tile([128, 2 * pad], F16, tag="xpn")
                nc.sync.dma_start(xpn_t[:], xpn_d[b])
                xpn_ts.append(xpn_t)

            # ---- CE + MSE (batched; Exp first so the Ln table loads once) --
            lg = mt_t[:, 0:40]
            pp = mt_t[:, 40:80]
            pn = mt_t[:, 80:120]
            tgb = mt_t[:, 120:160]
            e_t = spool.tile([128, 40], F32, tag="e")
            nc.scalar.activation(e_t[:], lg, ACTF.Exp)
            s1 = spool.tile([128, ST], F32, tag="s1")
            for k in range(ST):
                nc.vector.reduce_sum(
                    s1[:, k : k + 1], e_t[:, 10 * k : 10 * k + 10], axis=AX.X
                )
            ls4 = spool.tile([128, ST], F32, tag="ls4")
            lse_p = spool.tile([128, 1], F32, tag="lse")
            nc.scalar.activation(ls4[:], s1[:], ACTF.Ln, accum_out=lse_p[:])
            oh = spool.tile([128, 40], F32, tag="oh")
            nc.vector.tensor_tensor(oh[:], iota_t, tgb, op=ALU.is_equal)
            ohs = spool.tile([128, 40], F32, tag="ohs")
            pick_p = spool.tile([128, 1], F32, tag="pick")
            nc.vector.scalar_tensor_tensor(
                ohs[:], oh[:], 1.0, lg, op0=ALU.mult, op1=ALU.mult,
                accum_out=pick_p[:],
            )
            nc.vector.tensor_tensor(
                outa_t[:, 12:13], lse_p[:], pick_p[:], op=ALU.subtract
            )
            d_t = spool.tile([128, 40], F32, tag="d")
            nc.vector.scalar_tensor_tensor(
                d_t[:], pp, -1.0, pn, op0=ALU.add, op1=ALU.add
            )
            d2_t = spool.tile([128, 40], F32, tag="d2")
            nc.vector.scalar_tensor_tensor(
                d2_t[:], d_t[:], 1.0, d_t[:], op0=ALU.mult, op1=ALU.mult,
                accum_out=outa_t[:, 13:14],
            )

            # ---- per-graph sums on PE (needs only the DMA) ----
            ps_s = pss.tile([1, ST * 256], F32, tag="psum_s")
            for b in range(ST):
                xpn_t = xpn_ts[b]
                for h in range(2):
                    for k in range(nch):
                        nc.tensor.matmul(
                            ps_s[:, 256 * b + 128 * h : 256 * b + 128 * (h + 1)],
                            lhsT=onesc_t,
                            rhs=xpn_t[:, pad * h + 128 * k : pad * h + 128 * (k + 1)],
                            start=(k == 0), stop=(k == nch - 1),
                        )

            # rho rows (fp16, shipped so host reuses the exact device values)
            rho16s = []
            for b in range(ST):
                sp_sl = ps_s[:, 256 * b : 256 * b + 128]
                sn_sl = ps_s[:, 256 * b + 128 : 256 * b + 256]
                nc.vector.tensor_copy(
                    outs_t[:, 384 * b : 384 * b + 256], ps_s[:, 256 * b : 256 * b + 256]
                )
                t1 = spool.tile([1, 128], F32, tag="t1")
                nc.vector.tensor_scalar(
                    t1[:], sp_sl, EPS, 0.0, op0=ALU.add, op1=ALU.add
                )
                t2 = spool.tile([1, 128], F32, tag="t2")
                nc.vector.tensor_scalar(
                    t2[:], sn_sl, EPS, 0.0, op0=ALU.add, op1=ALU.add
                )
                t3 = spool.tile([1, 128], F32, tag="t3")
                nc.vector.reciprocal(t3[:], t2[:])
                rho = spool.tile([1, 128], F16, tag="rho")
                nc.vector.tensor_tensor(rho[:], t1[:], t3[:], op=ALU.mult)
                rho16s.append(rho)
                nc.vector.tensor_copy(outs_t[:, 384 * b + 256 : 384 * b + 384], rho[:])

            # ---- joint Ln over (xp|xn), raw values (dense ACT queue) ----
            lpn_ts = []
            for b in range(ST):
                lpn_t = lpool.tile([128, 2 * pad], F16, tag="lpn")
                nc.scalar.activation(
                    lpn_t[:], xpn_ts[b][:], ACTF.Ln, bias=eps_t, scale=1.0
                )
                lpn_ts.append(lpn_t)

            # ---- mixture y = xp + rho*xn (rho broadcast via K=1 matmuls;
            # two half-width passes to fit the PSUM bank budget) ----
            y_ts = []
            hw = pad // 2
            for b in range(ST):
                y_t = ypool.tile([128, pad], F16, tag="y")
                for h in range(2):
                    ps_r = psr.tile([128, hw], F32, tag="psum_rho")
                    for k in range(nch // 2):
                        nc.tensor.matmul(
                            ps_r[:, 128 * k : 128 * (k + 1)],
                            lhsT=onesr_t, rhs=rho16s[b][:],
                            start=True, stop=True,
                        )
                    sl = np.s_[:, h * hw : (h + 1) * hw]
                    tmp_t = ypool.tile([128, hw], F16, tag="tmp")
                    nc.vector.tensor_tensor(
                        tmp_t[:], xpn_ts[b][:, pad + h * hw : pad + (h + 1) * hw],
                        ps_r[:], op=ALU.mult,
                    )
                    nc.vector.tensor_tensor(
                        y_t[sl], tmp_t[:], xpn_ts[b][:, h * hw : (h + 1) * hw],
                        op=ALU.add,
                    )
                y_ts.append(y_t)

            # ---- Ln over the mixture (scale=0.5: ln(y/2 + eps)) ----
            lm_ts = []
            for b in range(ST):
                lm_t = ypool.tile([128, pad], F16, tag="lm")
                nc.scalar.activation(
                    lm_t[:], y_ts[b][:], ACTF.Ln, bias=eps_t, scale=0.5
                )
                lm_ts.append(lm_t)

            # ---- product-sums as PSUM-accumulated diagonals ----
            def emit_prod(src0, src1, col):
                ps_p = psp.tile([128, 128], F32, tag="psum_p")
                for k in range(nch):
                    nc.tensor.matmul(
                        ps_p[:], lhsT=src0[:, 128 * k : 128 * (k + 1)],
                        rhs=src1[:, 128 * k : 128 * (k + 1)],
                        start=(k == 0), stop=(k == nch - 1),
                    )
                scr = cpool.tile([128, 128], F16, tag="scr")
                nc.vector.scalar_tensor_tensor(
                    scr[:], ps_p[:], 1.0, iden_t[:, 0:128], op0=ALU.mult,
                    op1=ALU.mult, accum_out=outa_t[:, col : col + 1],
                )

            for b in range(ST):
                xpn_t = xpn_ts[b]
                emit_prod(xpn_t[:, 0:pad], lpn_ts[b][:, 0:pad], 0 + b)
                emit_prod(xpn_t[:, pad : 2 * pad], lpn_ts[b][:, pad : 2 * pad], 4 + b)
            for b in range(ST):
                emit_prod(y_ts[b][:], lm_ts[b][:], 8 + b)

            nc.sync.dma_start(outa_d[:], outa_t[:])
            nc.sync.dma_start(outs_d[:], outs_t[:])

    nc.finalize()
    return nc


def _pack_host(score_pos, score_neg, batch, pad):
    """Group nodes by graph, zero-pad to [NUM_GRAPHS, pad], fp16."""
    n = batch.shape[0]
    counts = np.bincount(batch, minlength=NUM_GRAPHS)
    assert counts.max() <= pad, f"graph size {counts.max()} exceeds pad {pad}"
    order = np.argsort(batch, kind="stable")
    bs = batch[order]
    starts = np.zeros(NUM_GRAPHS, np.int64)
    starts[1:] = np.cumsum(counts)[:-1]
    pos = np.arange(n, dtype=np.int64) - starts[bs]
    xp = np.zeros((NUM_GRAPHS, pad), np.float16)
    xn = np.zeros((NUM_GRAPHS, pad), np.float16)
    xp[bs, pos] = np.asarray(score_pos, np.float16)[order]
    xn[bs, pos] = np.asarray(score_neg, np.float16)[order]
    return xp, xn, counts


def _transpose_pack(x, pad):
    """[4096, pad] -> [NCORES, ST, 128, pad]: slot 128k+p -> (part p, col 128k+g)."""
    nch = pad // 128
    return (
        x.reshape(NCORES, ST, 128, nch, 128)
        .transpose(0, 1, 4, 3, 2)
        .reshape(NCORES, ST, 128, pad)
    )


_NC_CACHE: dict = {}


def kernel(logits_pos, probs_pos, probs_neg, score_pos, score_neg, targets, batch):
    global LAST_RESULTS
    logits_pos = np.asarray(logits_pos, np.float32)
    probs_pos = np.asarray(probs_pos, np.float32)
    probs_neg = np.asarray(probs_neg, np.float32)
    score_pos = np.asarray(score_pos, np.float32)
    score_neg = np.asarray(score_neg, np.float32)
    targets = np.asarray(targets)
    batch = np.asarray(batch)

    counts0 = np.bincount(batch, minlength=NUM_GRAPHS)
    pad = PAD if counts0.max() <= PAD else int(np.ceil(counts0.max() / 128) * 128)
    xp, xn, counts = _pack_host(score_pos, score_neg, batch, pad)
    xpn = np.concatenate(
        [_transpose_pack(xp, pad), _transpose_pack(xn, pad)], axis=-1
    )  # [c, b, 128, 2*pad]

    def fold(a):  # [4096, 10] -> [core, 128, 40]
        return (
            a.reshape(NCORES, ST, 128, NUM_CLASSES)
            .transpose(0, 2, 1, 3)
            .reshape(NCORES, 128, ST * NUM_CLASSES)
        )

    tgb = np.repeat(
        targets.astype(np.float32).reshape(NCORES, ST, 128, 1), NUM_CLASSES, axis=3
    )
    mt = np.concatenate(
        [fold(logits_pos), fold(probs_pos), fold(probs_neg), fold(tgb)], axis=2
    ).astype(np.float32)

    if pad not in _NC_CACHE:
        _NC_CACHE[pad] = _build_nc(pad)
    nc = _NC_CACHE[pad]

    in_maps = [{"xpn": xpn[c], "mt": mt[c]} for c in range(NCORES)]
    trace = bool(int(os.environ.get("KERNEL_TRACE", "0")))
    res = run_bass_kernel_spmd(nc, in_maps, list(range(NCORES)), trace=trace)
    LAST_RESULTS = res

    # --- host unshard: combine per-core per-graph partials in float64 ---
    outa = np.stack(
        [np.asarray(res.results[c]["outA"], np.float64) for c in range(NCORES)]
    )  # [core, 128, 14]
    outs = np.stack(
        [np.asarray(res.results[c]["outS"], np.float64) for c in range(NCORES)]
    ).reshape(NCORES, ST, 384)

    def graphs(colbase):  # [core, 128, ST] cols -> [4096] in graph order
        return outa[:, :, colbase : colbase + ST].transpose(0, 2, 1).reshape(-1)

    A_p, A_n, A_m = graphs(0), graphs(4), graphs(8)
    S_p = outs[:, :, 0:128].reshape(-1)
    S_n = outs[:, :, 128:256].reshape(-1)
    rho = outs[:, :, 256:384].reshape(-1)  # the device's fp16 rho, exact

    rp = 1.0 / (S_p + EPS)
    rn = 1.0 / (S_n + EPS)
    P = rp * A_p + (rp * S_p) * np.log(rp)
    N = rn * A_n + (rn * S_n) * np.log(rn)
    Sy = S_p + rho * S_n
    M = rp * A_m + (rp * Sy) * np.log(rp)
    kl = P + N - M

    num_graphs = float((counts > 0).sum())
    js = 0.5 * kl.sum() / num_graphs
    l_train = outa[:, :, 12].sum() / NUM_GRAPHS
    mse = outa[:, :, 13].sum() / (NUM_GRAPHS * NUM_CLASSES)
    l_cor = ALPHA * js + BETA * mse
    l_total = l_train + LAMBDA_COR * l_cor
    return (np.float32(l_total), np.float32(l_train), np.float32(l_cor))
